# revision 32
# baseline (speedup 1.0000x reference)
"""Depthwise causal conv1d (K=4) Trainium2 kernel.

Problem: x (B=8, S=4096, F=2048) f32, conv_state (B, F, 3), weight (F, 1, 4),
bias (F,).  out[b, s, f] = bias[f] + sum_d weight[f,0,d] * xpad[b, s+d, f]
where xpad = concat(conv_state[b].T, x[b]) along time.  Also returns
new_conv_state = xpad[:, -3:, :].T (host-side: it is a pure input slice).

Sharding: batch across the 8 NeuronCores (one item per core), weights
replicated.  No cross-device communication.

Per-core layout: S on partitions, F free (natural DMA: 8KB contiguous rows).
Two algorithms (ALGO flag):

"shift" (default, ~383us HW) -- per 125-row tile (128 input rows, 3-row
halo): y_d = x*w_d on DVE (4 tensor_tensor passes, f32->float32r), PE sums
the shifted taps via 0/1 Toeplitz float32r matmuls into PSUM (exact 0/1
weights; data pays only f32r rounding), ACT evacuates PSUM->SBUF.  Bias is
folded via slot-persistent rows: tiles 0-2 plant [0, 0, bias] in rows
125..127 of the three rotating y_0 pool slots (the d=0 multiply writes only
rows 0..124, so the data survives slot reuse) and W_0's all-ones row 127
adds it to every output row -- no bias matmuls.  Tile-0's x load is issued
before the weight constants (which ride the scalar HWDGE ring concurrently)
to shrink the kernel head.  ~1.0e-4 rel err.

"wino" (experimental, 487us: after fixing pipeline bubbles it is purely
PE-bound -- 16 bank-limited float32r matmuls/tile at ~630ns effective each
including per-matmul LDWEIGHTS/semaphore overhead) -- Winograd F(4,4): PE
applies banded data/output transforms, DVE does only 1.75 multiply passes.
~4.4e-4 rel err.

Engine notes baked in from profiling: GpSimd shares an SBUF read port with
DVE (tensor_tensor on both = mutual lock, net loss) so it is unused; big
DMAs must span all 128 partitions or the descriptor spread degenerates onto
one SDMA engine; float32r matmuls need every producer to emit float32r.
"""

import numpy as np
from fractions import Fraction

B, S, F, K = 8, 4096, 2048, 4
P = 128
CHUNK = 512  # PSUM bank = 512 fp32

ALGO = "shift"  # "wino" | "shift"

# Winograd F(4,4) geometry
M_BLK = 4
N_PTS = 7
POINTS = (0, 1, -1, 2, -2, Fraction(1, 2))
NBLK = 31
W_TILE_OUT = M_BLK * NBLK  # 124
S1_I = (0, 1, 2, 3)
S2_I = (4, 5, 6)
S1_ROWS = len(S1_I) * NBLK  # 124
S2_ROWS = len(S2_I) * NBLK + 1  # 94 (incl bias row)
BIAS_ROW = len(S2_I) * NBLK  # 93

_CACHE = {}
LAST_RESULTS = None
TRACE = False


# ---------------------------------------------------------------- winograd


def _wino_mats():
    n, m, r = N_PTS, M_BLK, K
    Vg = [[Fraction(0)] * r for _ in range(n)]
    Vd = [[Fraction(0)] * m for _ in range(n)]
    Vf = [[Fraction(0)] * n for _ in range(n)]
    for i, a in enumerate(POINTS):
        for j in range(r):
            Vg[i][j] = Fraction(a) ** j
        for j in range(m):
            Vd[i][j] = Fraction(a) ** j
        for j in range(n):
            Vf[i][j] = Fraction(a) ** j
    Vg[n - 1][r - 1] = Fraction(1)
    Vd[n - 1][m - 1] = Fraction(1)
    Vf[n - 1][n - 1] = Fraction(1)
    A = [row[:] + [Fraction(int(i == k)) for k in range(n)] for i, row in enumerate(Vf)]
    for col in range(n):
        piv = next(i for i in range(col, n) if A[i][col] != 0)
        A[col], A[piv] = A[piv], A[col]
        pv = A[col][col]
        A[col] = [v / pv for v in A[col]]
        for i in range(n):
            if i != col and A[i][col] != 0:
                f = A[i][col]
                A[i] = [v - f * w for v, w in zip(A[i], A[col])]
    L = [row[n:] for row in A]
    G = np.array([[float(v) for v in row] for row in Vg])
    AT = np.array([[float(v) for v in row] for row in Vd]).T  # (4, 7)
    BT = np.array([[float(v) for v in row] for row in L]).T  # (7, 7)
    return G, BT, AT


def _wino_big_mats(shift=0):
    """BigB1 (P, 124), BigB2 (P, 93 or 94), BigA1 (124, P), BigA2 (.., P).

    xt row 4b+j+shift feeds block b's j-th window slot.  With shift=0 the
    ones row sits at xt[127] and feeds the bias column (stack-2 col 93);
    with shift=1 (last tile) there is no bias column.
    """
    G, BT, AT = _wino_mats()
    s2cols = S2_ROWS if shift == 0 else S2_ROWS - 1
    BigB1 = np.zeros((P, S1_ROWS))
    BigB2 = np.zeros((P, s2cols))
    for b in range(NBLK):
        for j in range(N_PTS):
            k = 4 * b + j + shift
            for si, i in enumerate(S1_I):
                BigB1[k, si * NBLK + b] = BT[i, j]
            for si, i in enumerate(S2_I):
                BigB2[k, si * NBLK + b] = BT[i, j]
    if shift == 0:
        BigB2[P - 1, BIAS_ROW] = 1.0
    BigA1 = np.zeros((S1_ROWS, P))
    BigA2 = np.zeros((s2cols, P))
    for b in range(NBLK):
        for t in range(M_BLK):
            m = 4 * b + t
            for si, i in enumerate(S1_I):
                BigA1[si * NBLK + b, m] = AT[t, i]
            for si, i in enumerate(S2_I):
                BigA2[si * NBLK + b, m] = AT[t, i]
    if shift == 0:
        BigA2[BIAS_ROW, :] = 1.0
    return (m.astype(np.float32) for m in (BigB1, BigB2, BigA1, BigA2))


def _wino_wstacks(w, bias):
    G, _, _ = _wino_mats()
    Gw = (G @ w.T.astype(np.float64)).astype(np.float32)  # (7, F)
    ws1 = np.zeros((S1_ROWS, F), dtype=np.float32)
    ws2 = np.zeros((S2_ROWS, F), dtype=np.float32)
    for si, i in enumerate(S1_I):
        ws1[si * NBLK : (si + 1) * NBLK, :] = Gw[i][None, :]
    for si, i in enumerate(S2_I):
        ws2[si * NBLK : (si + 1) * NBLK, :] = Gw[i][None, :]
    ws2[BIAS_ROW, :] = bias
    return ws1, ws2


def _build_wino():
    import concourse.tile as tile
    import concourse.bacc as bacc
    from concourse import mybir

    f32 = mybir.dt.float32
    f32r = mybir.dt.float32r

    nc = bacc.Bacc("TRN2", target_bir_lowering=False, debug=False)

    x_dram = nc.dram_tensor("x", (S, F), f32, kind="ExternalInput")
    state_dram = nc.dram_tensor("state", (K - 1, F), f32, kind="ExternalInput")
    onesrow_dram = nc.dram_tensor("onesrow", (1, F), f32, kind="ExternalInput")
    bigb_dram = nc.dram_tensor(
        "bigb", (P, S1_ROWS + S2_ROWS), f32, kind="ExternalInput"
    )
    biga_dram = nc.dram_tensor(
        "biga", (S1_ROWS + S2_ROWS, P), f32, kind="ExternalInput"
    )
    bigbs_dram = nc.dram_tensor(
        "bigbs", (P, S1_ROWS + S2_ROWS - 1), f32, kind="ExternalInput"
    )
    wstack_dram = nc.dram_tensor(
        "wstack", (S1_ROWS + S2_ROWS, F), f32, kind="ExternalInput"
    )
    biasrow_dram = nc.dram_tensor("biasrow", (1, F), f32, kind="ExternalInput")
    ones_dram = nc.dram_tensor("ones", (1, P), f32, kind="ExternalInput")
    out_dram = nc.dram_tensor("out", (S, F), f32, kind="ExternalOutput")

    N_TILES = (S - 1) // W_TILE_OUT + 1  # 34; last tile overlaps, shifted B

    with tile.TileContext(nc) as tc:
        with (
            tc.tile_pool(name="consts", bufs=1) as consts,
            tc.tile_pool(name="xp", bufs=4) as xp,
            tc.tile_pool(name="xrp", bufs=3) as xrp,
            tc.tile_pool(name="prodp", bufs=4) as prodp,
            tc.tile_pool(name="op", bufs=4) as op,
            tc.tile_pool(name="psD", bufs=2, space="PSUM") as psD,
            tc.tile_pool(name="psA", bufs=4, space="PSUM") as psA,
        ):
            def const_f32r(name, shape, src_ap):
                t_f = consts.tile(list(shape), f32, tag=name + "_f")
                nc.sync.dma_start(t_f[:], src_ap)
                t_r = consts.tile(list(shape), f32r, tag=name)
                nc.vector.tensor_copy(t_r[:], t_f[:])
                return t_r

            bigb1 = const_f32r("bigb1", (P, S1_ROWS), bigb_dram.ap()[:, 0:S1_ROWS])
            bigb2 = const_f32r(
                "bigb2", (P, S2_ROWS), bigb_dram.ap()[:, S1_ROWS : S1_ROWS + S2_ROWS]
            )
            biga1 = const_f32r("biga1", (S1_ROWS, P), biga_dram.ap()[0:S1_ROWS])
            biga2 = const_f32r(
                "biga2", (S2_ROWS, P), biga_dram.ap()[S1_ROWS : S1_ROWS + S2_ROWS]
            )
            bigb1s = const_f32r("bigb1s", (P, S1_ROWS), bigbs_dram.ap()[:, 0:S1_ROWS])
            bigb2s = const_f32r(
                "bigb2s",
                (P, S2_ROWS - 1),
                bigbs_dram.ap()[:, S1_ROWS : S1_ROWS + S2_ROWS - 1],
            )
            ones = const_f32r("ones", (1, P), ones_dram.ap())
            biasrow = const_f32r("biasrow", (1, F), biasrow_dram.ap())

            ws1 = consts.tile([S1_ROWS, F], f32, tag="ws1")
            nc.sync.dma_start(ws1[:], wstack_dram.ap()[0:S1_ROWS])
            ws2 = consts.tile([S2_ROWS, F], f32, tag="ws2")
            nc.sync.dma_start(ws2[:], wstack_dram.ap()[S1_ROWS : S1_ROWS + S2_ROWS])

            def load_tile(j):
                # x load (always full 128 partitions) + ones row + f32r cast.
                # Called one tile ahead so the ACT cast lands before tile j's
                # evacuations in ACT's stream (software pipelining).
                last = j == N_TILES - 1
                r0 = min(W_TILE_OUT * j, S - W_TILE_OUT)
                xt = xp.tile([P, F], f32)
                if j == 0:
                    nc.sync.dma_start(xt[0 : K - 1, :], state_dram.ap())
                    nc.sync.dma_start(
                        xt[K - 1 : P - 1, :], x_dram.ap()[0 : P - K, :]
                    )
                elif last:
                    nc.sync.dma_start(xt[:], x_dram.ap()[r0 - K : r0 - K + P, :])
                else:
                    nc.sync.dma_start(
                        xt[:], x_dram.ap()[r0 - (K - 1) : r0 - (K - 1) + P, :]
                    )
                if not last:
                    nc.sync.dma_start(xt[P - 1 : P, :], onesrow_dram.ap())
                xr = xrp.tile([P, F], f32r)
                nc.scalar.copy(xr[:], xt[:])
                return xr

            xr_next = load_tile(0)
            for j in range(N_TILES):
                xr = xr_next
                if j + 1 < N_TILES:
                    xr_next = load_tile(j + 1)

                last = j == N_TILES - 1
                r0 = min(W_TILE_OUT * j, S - W_TILE_OUT)
                b1 = bigb1s if last else bigb1
                b2 = bigb2s if last else bigb2
                s2r = (S2_ROWS - 1) if last else S2_ROWS

                ot = op.tile([P, F], f32)
                for half in range(2):
                    cs = (2 * half, 2 * half + 1)
                    d1s, d2s, p1s, p2s, accs = {}, {}, {}, {}, {}
                    for c in cs:
                        sl = slice(CHUNK * c, CHUNK * (c + 1))
                        t_d1 = psD.tile([P, CHUNK], f32, tag="d1")
                        d1s[c] = t_d1
                        nc.tensor.matmul(
                            d1s[c][0:S1_ROWS, :], b1[:], xr[:, sl],
                            start=True, stop=True,
                        )
                    for c in cs:
                        sl = slice(CHUNK * c, CHUNK * (c + 1))
                        t_d2 = psD.tile([P, CHUNK], f32, tag="d2")
                        d2s[c] = t_d2
                        nc.tensor.matmul(
                            d2s[c][0:s2r, :], b2[:], xr[:, sl],
                            start=True, stop=True,
                        )
                    for c in cs:
                        sl = slice(CHUNK * c, CHUNK * (c + 1))
                        t_p1 = prodp.tile([P, CHUNK], f32r, tag="p1")
                        p1s[c] = t_p1
                        nc.vector.tensor_mul(
                            p1s[c][0:S1_ROWS, :], d1s[c][0:S1_ROWS, :], ws1[:, sl]
                        )
                        t_p2 = prodp.tile([P, CHUNK], f32r, tag="p2")
                        p2s[c] = t_p2
                        nc.vector.tensor_mul(
                            p2s[c][0:s2r, :], d2s[c][0:s2r, :], ws2[0:s2r, sl]
                        )
                    for c in cs:
                        t_acc = psA.tile([P, CHUNK], f32, tag="acc")
                        accs[c] = t_acc
                        nc.tensor.matmul(
                            accs[c][:, :], biga1[:], p1s[c][0:S1_ROWS, :],
                            start=True, stop=False,
                        )
                    for c in cs:
                        nc.tensor.matmul(
                            accs[c][:, :], biga2[0:s2r, :], p2s[c][0:s2r, :],
                            start=False, stop=not last,
                        )
                    if last:
                        for c in cs:
                            sl = slice(CHUNK * c, CHUNK * (c + 1))
                            nc.tensor.matmul(
                                accs[c][:, :], ones[:], biasrow[:, sl],
                                start=False, stop=True,
                            )
                    for c in cs:
                        sl = slice(CHUNK * c, CHUNK * (c + 1))
                        nc.scalar.copy(
                            ot[0:W_TILE_OUT, sl], accs[c][0:W_TILE_OUT, :]
                        )

                nc.scalar.dma_start(
                    out_dram.ap()[r0 : r0 + W_TILE_OUT, :], ot[0:W_TILE_OUT, :]
                )

    nc.compile()
    return nc


def _wino_consts_np(weight, bias):
    w = weight[:, 0, :].astype(np.float32)  # (F, K)
    B1, B2, A1, A2 = _wino_big_mats(shift=0)
    B1s, B2s, _, _ = _wino_big_mats(shift=1)
    bigb = np.concatenate([B1, B2], axis=1)
    biga = np.concatenate([A1, A2], axis=0)
    bigbs = np.concatenate([B1s, B2s], axis=1)
    ws1, ws2 = _wino_wstacks(w, bias)
    wstack = np.concatenate([ws1, ws2], axis=0)
    return {
        "bigb": np.ascontiguousarray(bigb),
        "biga": np.ascontiguousarray(biga),
        "bigbs": np.ascontiguousarray(bigbs),
        "wstack": np.ascontiguousarray(wstack),
        "biasrow": np.ascontiguousarray(bias[None, :], dtype=np.float32),
        "ones": np.ones((1, P), dtype=np.float32),
        "onesrow": np.ones((1, F), dtype=np.float32),
    }


# ------------------------------------------------------------------- shift


def _enable_ldw_opt():
    import concourse.bass_utils as _bu

    if getattr(_bu, "_ldw_patched", False):
        return
    _orig = _bu.run_command

    def _rc(argv, **kw):
        return _orig(argv, **kw)

    _bu.run_command = _rc
    _bu._ldw_patched = True


def _build_shift():
    import concourse.tile as tile
    import concourse.bacc as bacc
    from concourse import mybir

    _enable_ldw_opt()

    f32 = mybir.dt.float32
    f32r = mybir.dt.float32r
    bf16 = mybir.dt.bfloat16

    TILE_OUT = 125
    N_TILES = (S + TILE_OUT - 1) // TILE_OUT
    NCH = F // CHUNK

    nc = bacc.Bacc("TRN2", target_bir_lowering=False, debug=False)

    x_dram = nc.dram_tensor("x", (S, F), f32, kind="ExternalInput")
    state_dram = nc.dram_tensor("state", (K - 1, F), f32, kind="ExternalInput")
    wrep_dram = nc.dram_tensor("wrep", (P, K, F), f32, kind="ExternalInput")
    biasinit_dram = nc.dram_tensor("biasinit", (K - 1, F), f32r, kind="ExternalInput")
    wshift_dram = nc.dram_tensor("wshift", (K, P, P), f32, kind="ExternalInput")
    biasrow_dram = nc.dram_tensor("biasrow", (1, F), f32, kind="ExternalInput")
    ones_dram = nc.dram_tensor("ones", (1, P), f32, kind="ExternalInput")
    onesrow_dram = nc.dram_tensor("onesrow", (1, F), f32, kind="ExternalInput")

    out_dram = nc.dram_tensor("out", (S, F), f32, kind="ExternalOutput")

    with tile.TileContext(nc) as tc:
        with (
            tc.tile_pool(name="consts", bufs=1) as consts,
            tc.tile_pool(name="xp", bufs=4) as xp,
            tc.tile_pool(name="yp", bufs=3) as yp,
            tc.tile_pool(name="op", bufs=3) as op,
            tc.tile_pool(name="psum", bufs=2, space="PSUM") as pp,
        ):
            # tile-0 x load first: nothing queues ahead of it on the sync
            # ring, so the first multiply starts as early as possible.
            xt0 = xp.tile([P, F], f32, tag="xt")
            nc.sync.dma_start(xt0[0 : K - 1, :], state_dram.ap())
            nc.sync.dma_start(xt0[K - 1 : P, :], x_dram.ap()[0:TILE_OUT, :])
            # Constants ride the scalar HWDGE ring, concurrent with the x
            # loads.  The tiny wshift tensors go FIRST so their f32r casts
            # (which sit ahead of the multiplies in DVE program order) are
            # unblocked immediately; the 4MB wrep follows.
            wshift = []
            for d in range(K):
                wsf = consts.tile([P, P], f32, tag=f"wshiftf{d}")
                nc.scalar.dma_start(wsf[:], wshift_dram.ap()[d])
                ws = consts.tile([P, P], f32r, tag=f"wshift{d}")
                nc.vector.tensor_copy(ws[:], wsf[:])
                wshift.append(ws)
            wrep = []
            for d in range(K):
                wr = consts.tile([P, F], f32, tag=f"wrep{d}")
                nc.scalar.dma_start(wr[:], wrep_dram.ap()[:, d, :])
                wrep.append(wr)
            biasrow = None
            ones = None

            for j in range(N_TILES):
                r0 = TILE_OUT * j
                n_out = min(TILE_OUT, S - r0)
                n_in = n_out + (K - 1)

                full = n_out == TILE_OUT
                if j == 0:
                    xt = xt0
                else:
                    xt = xp.tile([P, F], f32, tag="xt")
                    nc.sync.dma_start(
                        xt[0:n_in, :], x_dram.ap()[r0 - (K - 1) : r0 + n_out, :]
                    )

                ys = []
                for d in range(K):
                    y = yp.tile([P, F], f32r, tag=f"y{d}")
                    # d=0 writes only the n_out rows the identity band reads;
                    # rows 125..127 of the three rotating y0 slots keep the
                    # [0, 0, bias] block planted below, which W_0's all-ones
                    # row 127 adds to every output row (bias for free).
                    rows = n_out if (full and d == 0) else n_in
                    nc.vector.tensor_mul(
                        y[0:rows, :], xt[0:rows, :], wrep[d][0:rows, :]
                    )
                    ys.append(y)
                if full and j < 3:
                    nc.sync.dma_start(ys[0][TILE_OUT:P, :], biasinit_dram.ap())
                if j == 1:
                    # biasrow/ones consts are only needed by the last tile's
                    # explicit bias matmuls; deferring them keeps their DMAs
                    # and DVE casts off the kernel head.
                    biasrow_f = consts.tile([1, F], f32)
                    nc.scalar.dma_start(biasrow_f[:], biasrow_dram.ap())
                    biasrow = consts.tile([1, F], f32r, tag="biasrow_r")
                    nc.vector.tensor_copy(biasrow[:], biasrow_f[:])
                    ones_f = consts.tile([1, P], f32)
                    nc.scalar.dma_start(ones_f[:], ones_dram.ap())
                    ones = consts.tile([1, P], f32r, tag="ones_r")
                    nc.vector.tensor_copy(ones[:], ones_f[:])

                acc = pp.tile([P, F], f32)
                for d in range(K):
                    for c in range(NCH):
                        sl = slice(CHUNK * c, CHUNK * (c + 1))
                        nc.tensor.matmul(
                            acc[:, sl],
                            wshift[d][0 : (P if (full and d == 0) else n_in), :],
                            ys[d][0 : (P if (full and d == 0) else n_in), sl],
                            start=(d == 0),
                            stop=(full and d == K - 1),
                        )
                if not full:
                    for c in range(NCH):
                        sl = slice(CHUNK * c, CHUNK * (c + 1))
                        nc.tensor.matmul(
                            acc[:, sl], ones[:], biasrow[:, sl],
                            start=False, stop=True,
                        )

                ot = op.tile([TILE_OUT, F], f32)
                nc.scalar.copy(ot[0:n_out, :], acc[0:n_out, :])
                # out-DMA on the ACT HWDGE ring (qActDynamicHW) so stores do
                # not queue behind the next tile's load on the SP ring FIFO.
                nc.scalar.dma_start(
                    out_dram.ap()[r0 : r0 + n_out, :], ot[0:n_out, :]
                )

    nc.compile()
    return nc


def _shift_consts_np(weight, bias):
    w = weight[:, 0, :].astype(np.float32)
    wrep = np.ascontiguousarray(
        np.broadcast_to(w.T[None, :, :], (P, K, F)), dtype=np.float32
    )
    wshift = np.zeros((K, P, P), dtype=np.float32)
    for d in range(K):
        for m in range(P - d):
            wshift[d, m + d, m] = 1.0
    # bias fold: W_0 row 127 = ones reads the planted bias row; diag entries
    # for the stale rows 125/126 are cleared so they contribute nothing.
    wshift[0, 125, 125] = 0.0
    wshift[0, 126, 126] = 0.0
    wshift[0, 127, :] = 1.0
    biasinit = np.zeros((K - 1, F), dtype=np.float32)
    biasinit[K - 2, :] = bias
    return {
        "wrep": wrep,
        "biasinit": biasinit,
        "wshift": wshift,
        "biasrow": np.ascontiguousarray(bias[None, :], dtype=np.float32),
        "ones": np.ones((1, P), dtype=np.float32),
        "onesrow": np.ones((1, F), dtype=np.float32),
    }


# -------------------------------------------------------------------- main


def kernel(x, conv_state, weight, bias):
    global LAST_RESULTS
    from concourse.bass_utils import run_bass_kernel_spmd

    x = np.asarray(x, dtype=np.float32)
    conv_state = np.asarray(conv_state, dtype=np.float32)
    weight = np.asarray(weight, dtype=np.float32)
    bias = np.asarray(bias, dtype=np.float32)

    key = "nc_" + ALGO
    if key not in _CACHE:
        _CACHE[key] = _build_wino() if ALGO == "wino" else _build_shift()
    nc = _CACHE[key]

    consts = (
        _wino_consts_np(weight, bias)
        if ALGO == "wino"
        else _shift_consts_np(weight, bias)
    )
    in_maps = []
    for b in range(B):
        m = {
            "x": np.ascontiguousarray(x[b]),
            "state": np.ascontiguousarray(conv_state[b].T),
        }
        m.update(consts)
        in_maps.append(m)

    kwargs = {}
    if TRACE:
        kwargs = dict(trace=True, trace_cores=[0])
    res = run_bass_kernel_spmd(nc, in_maps, core_ids=list(range(B)), **kwargs)
    LAST_RESULTS = res

    out = np.stack([res.results[b]["out"] for b in range(B)], axis=0)
    new_conv_state = np.ascontiguousarray(
        x[:, S - (K - 1) :, :].transpose(0, 2, 1), dtype=np.float32
    )
    return out, new_conv_state


# revision 36
# speedup vs baseline: 1.0066x; 1.0066x over previous
"""Depthwise causal conv1d (K=4) Trainium2 kernel.

Problem: x (B=8, S=4096, F=2048) f32, conv_state (B, F, 3), weight (F, 1, 4),
bias (F,).  out[b, s, f] = bias[f] + sum_d weight[f,0,d] * xpad[b, s+d, f]
where xpad = concat(conv_state[b].T, x[b]) along time.  Also returns
new_conv_state = xpad[:, -3:, :].T (host-side: it is a pure input slice).

Sharding: batch across the 8 NeuronCores (one item per core), weights
replicated.  No cross-device communication.

Per-core layout: S on partitions, F free (natural DMA: 8KB contiguous rows).
Two algorithms (ALGO flag):

"shift" (default, ~383us HW) -- per 125-row tile (128 input rows, 3-row
halo): y_d = x*w_d on DVE (4 tensor_tensor passes, f32->float32r), PE sums
the shifted taps via 0/1 Toeplitz float32r matmuls into PSUM (exact 0/1
weights; data pays only f32r rounding), ACT evacuates PSUM->SBUF.  Bias is
folded via slot-persistent rows: tiles 0-2 plant [0, 0, bias] in rows
125..127 of the three rotating y_0 pool slots (the d=0 multiply writes only
rows 0..124, so the data survives slot reuse) and W_0's all-ones row 127
adds it to every output row -- no bias matmuls.  Tile-0's x load is issued
before the weight constants (which ride the scalar HWDGE ring concurrently)
to shrink the kernel head.  ~1.0e-4 rel err.

"wino" (experimental, 487us: after fixing pipeline bubbles it is purely
PE-bound -- 16 bank-limited float32r matmuls/tile at ~630ns effective each
including per-matmul LDWEIGHTS/semaphore overhead) -- Winograd F(4,4): PE
applies banded data/output transforms, DVE does only 1.75 multiply passes.
~4.4e-4 rel err.

Engine notes baked in from profiling: GpSimd shares an SBUF read port with
DVE (tensor_tensor on both = mutual lock, net loss) so it is unused; big
DMAs must span all 128 partitions or the descriptor spread degenerates onto
one SDMA engine; float32r matmuls need every producer to emit float32r.
"""

import numpy as np
from fractions import Fraction

B, S, F, K = 8, 4096, 2048, 4
P = 128
CHUNK = 512  # PSUM bank = 512 fp32

ALGO = "shift"  # "wino" | "shift"

# Winograd F(4,4) geometry
M_BLK = 4
N_PTS = 7
POINTS = (0, 1, -1, 2, -2, Fraction(1, 2))
NBLK = 31
W_TILE_OUT = M_BLK * NBLK  # 124
S1_I = (0, 1, 2, 3)
S2_I = (4, 5, 6)
S1_ROWS = len(S1_I) * NBLK  # 124
S2_ROWS = len(S2_I) * NBLK + 1  # 94 (incl bias row)
BIAS_ROW = len(S2_I) * NBLK  # 93

_CACHE = {}
LAST_RESULTS = None
TRACE = False


# ---------------------------------------------------------------- winograd


def _wino_mats():
    n, m, r = N_PTS, M_BLK, K
    Vg = [[Fraction(0)] * r for _ in range(n)]
    Vd = [[Fraction(0)] * m for _ in range(n)]
    Vf = [[Fraction(0)] * n for _ in range(n)]
    for i, a in enumerate(POINTS):
        for j in range(r):
            Vg[i][j] = Fraction(a) ** j
        for j in range(m):
            Vd[i][j] = Fraction(a) ** j
        for j in range(n):
            Vf[i][j] = Fraction(a) ** j
    Vg[n - 1][r - 1] = Fraction(1)
    Vd[n - 1][m - 1] = Fraction(1)
    Vf[n - 1][n - 1] = Fraction(1)
    A = [row[:] + [Fraction(int(i == k)) for k in range(n)] for i, row in enumerate(Vf)]
    for col in range(n):
        piv = next(i for i in range(col, n) if A[i][col] != 0)
        A[col], A[piv] = A[piv], A[col]
        pv = A[col][col]
        A[col] = [v / pv for v in A[col]]
        for i in range(n):
            if i != col and A[i][col] != 0:
                f = A[i][col]
                A[i] = [v - f * w for v, w in zip(A[i], A[col])]
    L = [row[n:] for row in A]
    G = np.array([[float(v) for v in row] for row in Vg])
    AT = np.array([[float(v) for v in row] for row in Vd]).T  # (4, 7)
    BT = np.array([[float(v) for v in row] for row in L]).T  # (7, 7)
    return G, BT, AT


def _wino_big_mats(shift=0):
    """BigB1 (P, 124), BigB2 (P, 93 or 94), BigA1 (124, P), BigA2 (.., P).

    xt row 4b+j+shift feeds block b's j-th window slot.  With shift=0 the
    ones row sits at xt[127] and feeds the bias column (stack-2 col 93);
    with shift=1 (last tile) there is no bias column.
    """
    G, BT, AT = _wino_mats()
    s2cols = S2_ROWS if shift == 0 else S2_ROWS - 1
    BigB1 = np.zeros((P, S1_ROWS))
    BigB2 = np.zeros((P, s2cols))
    for b in range(NBLK):
        for j in range(N_PTS):
            k = 4 * b + j + shift
            for si, i in enumerate(S1_I):
                BigB1[k, si * NBLK + b] = BT[i, j]
            for si, i in enumerate(S2_I):
                BigB2[k, si * NBLK + b] = BT[i, j]
    if shift == 0:
        BigB2[P - 1, BIAS_ROW] = 1.0
    BigA1 = np.zeros((S1_ROWS, P))
    BigA2 = np.zeros((s2cols, P))
    for b in range(NBLK):
        for t in range(M_BLK):
            m = 4 * b + t
            for si, i in enumerate(S1_I):
                BigA1[si * NBLK + b, m] = AT[t, i]
            for si, i in enumerate(S2_I):
                BigA2[si * NBLK + b, m] = AT[t, i]
    if shift == 0:
        BigA2[BIAS_ROW, :] = 1.0
    return (m.astype(np.float32) for m in (BigB1, BigB2, BigA1, BigA2))


def _wino_wstacks(w, bias):
    G, _, _ = _wino_mats()
    Gw = (G @ w.T.astype(np.float64)).astype(np.float32)  # (7, F)
    ws1 = np.zeros((S1_ROWS, F), dtype=np.float32)
    ws2 = np.zeros((S2_ROWS, F), dtype=np.float32)
    for si, i in enumerate(S1_I):
        ws1[si * NBLK : (si + 1) * NBLK, :] = Gw[i][None, :]
    for si, i in enumerate(S2_I):
        ws2[si * NBLK : (si + 1) * NBLK, :] = Gw[i][None, :]
    ws2[BIAS_ROW, :] = bias
    return ws1, ws2


def _build_wino():
    import concourse.tile as tile
    import concourse.bacc as bacc
    from concourse import mybir

    f32 = mybir.dt.float32
    f32r = mybir.dt.float32r

    nc = bacc.Bacc("TRN2", target_bir_lowering=False, debug=False)

    x_dram = nc.dram_tensor("x", (S, F), f32, kind="ExternalInput")
    state_dram = nc.dram_tensor("state", (K - 1, F), f32, kind="ExternalInput")
    onesrow_dram = nc.dram_tensor("onesrow", (1, F), f32, kind="ExternalInput")
    bigb_dram = nc.dram_tensor(
        "bigb", (P, S1_ROWS + S2_ROWS), f32, kind="ExternalInput"
    )
    biga_dram = nc.dram_tensor(
        "biga", (S1_ROWS + S2_ROWS, P), f32, kind="ExternalInput"
    )
    bigbs_dram = nc.dram_tensor(
        "bigbs", (P, S1_ROWS + S2_ROWS - 1), f32, kind="ExternalInput"
    )
    wstack_dram = nc.dram_tensor(
        "wstack", (S1_ROWS + S2_ROWS, F), f32, kind="ExternalInput"
    )
    biasrow_dram = nc.dram_tensor("biasrow", (1, F), f32, kind="ExternalInput")
    ones_dram = nc.dram_tensor("ones", (1, P), f32, kind="ExternalInput")
    out_dram = nc.dram_tensor("out", (S, F), f32, kind="ExternalOutput")

    N_TILES = (S - 1) // W_TILE_OUT + 1  # 34; last tile overlaps, shifted B

    with tile.TileContext(nc) as tc:
        with (
            tc.tile_pool(name="consts", bufs=1) as consts,
            tc.tile_pool(name="xp", bufs=4) as xp,
            tc.tile_pool(name="xrp", bufs=3) as xrp,
            tc.tile_pool(name="prodp", bufs=4) as prodp,
            tc.tile_pool(name="op", bufs=4) as op,
            tc.tile_pool(name="psD", bufs=2, space="PSUM") as psD,
            tc.tile_pool(name="psA", bufs=4, space="PSUM") as psA,
        ):
            def const_f32r(name, shape, src_ap):
                t_f = consts.tile(list(shape), f32, tag=name + "_f")
                nc.sync.dma_start(t_f[:], src_ap)
                t_r = consts.tile(list(shape), f32r, tag=name)
                nc.vector.tensor_copy(t_r[:], t_f[:])
                return t_r

            bigb1 = const_f32r("bigb1", (P, S1_ROWS), bigb_dram.ap()[:, 0:S1_ROWS])
            bigb2 = const_f32r(
                "bigb2", (P, S2_ROWS), bigb_dram.ap()[:, S1_ROWS : S1_ROWS + S2_ROWS]
            )
            biga1 = const_f32r("biga1", (S1_ROWS, P), biga_dram.ap()[0:S1_ROWS])
            biga2 = const_f32r(
                "biga2", (S2_ROWS, P), biga_dram.ap()[S1_ROWS : S1_ROWS + S2_ROWS]
            )
            bigb1s = const_f32r("bigb1s", (P, S1_ROWS), bigbs_dram.ap()[:, 0:S1_ROWS])
            bigb2s = const_f32r(
                "bigb2s",
                (P, S2_ROWS - 1),
                bigbs_dram.ap()[:, S1_ROWS : S1_ROWS + S2_ROWS - 1],
            )
            ones = const_f32r("ones", (1, P), ones_dram.ap())
            biasrow = const_f32r("biasrow", (1, F), biasrow_dram.ap())

            ws1 = consts.tile([S1_ROWS, F], f32, tag="ws1")
            nc.sync.dma_start(ws1[:], wstack_dram.ap()[0:S1_ROWS])
            ws2 = consts.tile([S2_ROWS, F], f32, tag="ws2")
            nc.sync.dma_start(ws2[:], wstack_dram.ap()[S1_ROWS : S1_ROWS + S2_ROWS])

            def load_tile(j):
                # x load (always full 128 partitions) + ones row + f32r cast.
                # Called one tile ahead so the ACT cast lands before tile j's
                # evacuations in ACT's stream (software pipelining).
                last = j == N_TILES - 1
                r0 = min(W_TILE_OUT * j, S - W_TILE_OUT)
                xt = xp.tile([P, F], f32)
                if j == 0:
                    nc.sync.dma_start(xt[0 : K - 1, :], state_dram.ap())
                    nc.sync.dma_start(
                        xt[K - 1 : P - 1, :], x_dram.ap()[0 : P - K, :]
                    )
                elif last:
                    nc.sync.dma_start(xt[:], x_dram.ap()[r0 - K : r0 - K + P, :])
                else:
                    nc.sync.dma_start(
                        xt[:], x_dram.ap()[r0 - (K - 1) : r0 - (K - 1) + P, :]
                    )
                if not last:
                    nc.sync.dma_start(xt[P - 1 : P, :], onesrow_dram.ap())
                xr = xrp.tile([P, F], f32r)
                nc.scalar.copy(xr[:], xt[:])
                return xr

            xr_next = load_tile(0)
            for j in range(N_TILES):
                xr = xr_next
                if j + 1 < N_TILES:
                    xr_next = load_tile(j + 1)

                last = j == N_TILES - 1
                r0 = min(W_TILE_OUT * j, S - W_TILE_OUT)
                b1 = bigb1s if last else bigb1
                b2 = bigb2s if last else bigb2
                s2r = (S2_ROWS - 1) if last else S2_ROWS

                ot = op.tile([P, F], f32)
                for half in range(2):
                    cs = (2 * half, 2 * half + 1)
                    d1s, d2s, p1s, p2s, accs = {}, {}, {}, {}, {}
                    for c in cs:
                        sl = slice(CHUNK * c, CHUNK * (c + 1))
                        t_d1 = psD.tile([P, CHUNK], f32, tag="d1")
                        d1s[c] = t_d1
                        nc.tensor.matmul(
                            d1s[c][0:S1_ROWS, :], b1[:], xr[:, sl],
                            start=True, stop=True,
                        )
                    for c in cs:
                        sl = slice(CHUNK * c, CHUNK * (c + 1))
                        t_d2 = psD.tile([P, CHUNK], f32, tag="d2")
                        d2s[c] = t_d2
                        nc.tensor.matmul(
                            d2s[c][0:s2r, :], b2[:], xr[:, sl],
                            start=True, stop=True,
                        )
                    for c in cs:
                        sl = slice(CHUNK * c, CHUNK * (c + 1))
                        t_p1 = prodp.tile([P, CHUNK], f32r, tag="p1")
                        p1s[c] = t_p1
                        nc.vector.tensor_mul(
                            p1s[c][0:S1_ROWS, :], d1s[c][0:S1_ROWS, :], ws1[:, sl]
                        )
                        t_p2 = prodp.tile([P, CHUNK], f32r, tag="p2")
                        p2s[c] = t_p2
                        nc.vector.tensor_mul(
                            p2s[c][0:s2r, :], d2s[c][0:s2r, :], ws2[0:s2r, sl]
                        )
                    for c in cs:
                        t_acc = psA.tile([P, CHUNK], f32, tag="acc")
                        accs[c] = t_acc
                        nc.tensor.matmul(
                            accs[c][:, :], biga1[:], p1s[c][0:S1_ROWS, :],
                            start=True, stop=False,
                        )
                    for c in cs:
                        nc.tensor.matmul(
                            accs[c][:, :], biga2[0:s2r, :], p2s[c][0:s2r, :],
                            start=False, stop=not last,
                        )
                    if last:
                        for c in cs:
                            sl = slice(CHUNK * c, CHUNK * (c + 1))
                            nc.tensor.matmul(
                                accs[c][:, :], ones[:], biasrow[:, sl],
                                start=False, stop=True,
                            )
                    for c in cs:
                        sl = slice(CHUNK * c, CHUNK * (c + 1))
                        nc.scalar.copy(
                            ot[0:W_TILE_OUT, sl], accs[c][0:W_TILE_OUT, :]
                        )

                nc.scalar.dma_start(
                    out_dram.ap()[r0 : r0 + W_TILE_OUT, :], ot[0:W_TILE_OUT, :]
                )

    nc.compile()
    return nc


def _wino_consts_np(weight, bias):
    w = weight[:, 0, :].astype(np.float32)  # (F, K)
    B1, B2, A1, A2 = _wino_big_mats(shift=0)
    B1s, B2s, _, _ = _wino_big_mats(shift=1)
    bigb = np.concatenate([B1, B2], axis=1)
    biga = np.concatenate([A1, A2], axis=0)
    bigbs = np.concatenate([B1s, B2s], axis=1)
    ws1, ws2 = _wino_wstacks(w, bias)
    wstack = np.concatenate([ws1, ws2], axis=0)
    return {
        "bigb": np.ascontiguousarray(bigb),
        "biga": np.ascontiguousarray(biga),
        "bigbs": np.ascontiguousarray(bigbs),
        "wstack": np.ascontiguousarray(wstack),
        "biasrow": np.ascontiguousarray(bias[None, :], dtype=np.float32),
        "ones": np.ones((1, P), dtype=np.float32),
        "onesrow": np.ones((1, F), dtype=np.float32),
    }


# ------------------------------------------------------------------- shift


def _enable_ldw_opt():
    import concourse.bass_utils as _bu

    if getattr(_bu, "_ldw_patched", False):
        return
    _orig = _bu.run_command

    def _rc(argv, **kw):
        return _orig(argv, **kw)

    _bu.run_command = _rc
    _bu._ldw_patched = True


def _build_shift():
    import concourse.tile as tile
    import concourse.bacc as bacc
    from concourse import mybir

    _enable_ldw_opt()

    f32 = mybir.dt.float32
    f32r = mybir.dt.float32r
    bf16 = mybir.dt.bfloat16

    TILE_OUT = 125
    N_TILES = (S + TILE_OUT - 1) // TILE_OUT
    NCH = F // CHUNK

    nc = bacc.Bacc("TRN2", target_bir_lowering=False, debug=False)

    x_dram = nc.dram_tensor("x", (S, F), f32, kind="ExternalInput")
    state_dram = nc.dram_tensor("state", (K - 1, F), f32, kind="ExternalInput")
    wrep_dram = nc.dram_tensor("wrep", (P, K, F), f32, kind="ExternalInput")
    biasinit_dram = nc.dram_tensor("biasinit", (K - 1, F), f32r, kind="ExternalInput")
    wshift_dram = nc.dram_tensor("wshift", (K, P, P), f32, kind="ExternalInput")
    biasrow_dram = nc.dram_tensor("biasrow", (1, F), f32, kind="ExternalInput")
    ones_dram = nc.dram_tensor("ones", (1, P), f32, kind="ExternalInput")
    onesrow_dram = nc.dram_tensor("onesrow", (1, F), f32, kind="ExternalInput")

    out_dram = nc.dram_tensor("out", (S, F), f32, kind="ExternalOutput")

    with tile.TileContext(nc) as tc:
        with (
            tc.tile_pool(name="consts", bufs=1) as consts,
            tc.tile_pool(name="xp", bufs=3) as xp,
            tc.tile_pool(name="yp", bufs=3) as yp,
            tc.tile_pool(name="op", bufs=4) as op,
            tc.tile_pool(name="psum", bufs=2, space="PSUM") as pp,
        ):
            # tile-0 x load first: nothing queues ahead of it on the sync
            # ring, so the first multiply starts as early as possible.
            xt0 = xp.tile([P, F], f32, tag="xt")
            nc.sync.dma_start(xt0[0 : K - 1, :], state_dram.ap())
            nc.sync.dma_start(xt0[K - 1 : P, :], x_dram.ap()[0:TILE_OUT, :])
            # Constants ride the scalar HWDGE ring, concurrent with the x
            # loads.  The tiny wshift tensors go FIRST so their f32r casts
            # (which sit ahead of the multiplies in DVE program order) are
            # unblocked immediately; the 4MB wrep follows.
            wshift = []
            for d in range(K):
                wsf = consts.tile([P, P], f32, tag=f"wshiftf{d}")
                nc.scalar.dma_start(wsf[:], wshift_dram.ap()[d])
                ws = consts.tile([P, P], f32r, tag=f"wshift{d}")
                nc.vector.tensor_copy(ws[:], wsf[:])
                wshift.append(ws)
            wrep = []
            for d in range(K):
                wr = consts.tile([P, F], f32, tag=f"wrep{d}")
                nc.scalar.dma_start(wr[:], wrep_dram.ap()[:, d, :])
                wrep.append(wr)
            biasrow = None
            ones = None

            for j in range(N_TILES):
                r0 = TILE_OUT * j
                n_out = min(TILE_OUT, S - r0)
                n_in = n_out + (K - 1)

                full = n_out == TILE_OUT
                if j == 0:
                    xt = xt0
                else:
                    xt = xp.tile([P, F], f32, tag="xt")
                    nc.sync.dma_start(
                        xt[0:n_in, :], x_dram.ap()[r0 - (K - 1) : r0 + n_out, :]
                    )

                ys = []
                for d in range(K):
                    y = yp.tile([P, F], f32r, tag=f"y{d}")
                    # d=0 writes only the n_out rows the identity band reads;
                    # rows 125..127 of the three rotating y0 slots keep the
                    # [0, 0, bias] block planted below, which W_0's all-ones
                    # row 127 adds to every output row (bias for free).
                    rows = n_out if (full and d == 0) else n_in
                    nc.vector.tensor_mul(
                        y[0:rows, :], xt[0:rows, :], wrep[d][0:rows, :]
                    )
                    ys.append(y)
                if full and j < 3:
                    nc.sync.dma_start(ys[0][TILE_OUT:P, :], biasinit_dram.ap())
                if j == 1:
                    # biasrow/ones consts are only needed by the last tile's
                    # explicit bias matmuls; deferring them keeps their DMAs
                    # and DVE casts off the kernel head.
                    biasrow_f = consts.tile([1, F], f32)
                    nc.scalar.dma_start(biasrow_f[:], biasrow_dram.ap())
                    biasrow = consts.tile([1, F], f32r, tag="biasrow_r")
                    nc.vector.tensor_copy(biasrow[:], biasrow_f[:])
                    ones_f = consts.tile([1, P], f32)
                    nc.scalar.dma_start(ones_f[:], ones_dram.ap())
                    ones = consts.tile([1, P], f32r, tag="ones_r")
                    nc.vector.tensor_copy(ones[:], ones_f[:])

                acc = pp.tile([P, F], f32)
                for d in range(K):
                    for c in range(NCH):
                        sl = slice(CHUNK * c, CHUNK * (c + 1))
                        nc.tensor.matmul(
                            acc[:, sl],
                            wshift[d][0 : (P if (full and d == 0) else n_in), :],
                            ys[d][0 : (P if (full and d == 0) else n_in), sl],
                            start=(d == 0),
                            stop=(full and d == K - 1),
                        )
                if not full:
                    for c in range(NCH):
                        sl = slice(CHUNK * c, CHUNK * (c + 1))
                        nc.tensor.matmul(
                            acc[:, sl], ones[:], biasrow[:, sl],
                            start=False, stop=True,
                        )

                ot = op.tile([TILE_OUT, F], f32)
                nc.scalar.copy(ot[0:n_out, :], acc[0:n_out, :])
                # out-DMA on the ACT HWDGE ring (qActDynamicHW) so stores do
                # not queue behind the next tile's load on the SP ring FIFO.
                nc.scalar.dma_start(
                    out_dram.ap()[r0 : r0 + n_out, :], ot[0:n_out, :]
                )

    nc.compile()
    return nc


def _shift_consts_np(weight, bias):
    w = weight[:, 0, :].astype(np.float32)
    wrep = np.ascontiguousarray(
        np.broadcast_to(w.T[None, :, :], (P, K, F)), dtype=np.float32
    )
    wshift = np.zeros((K, P, P), dtype=np.float32)
    for d in range(K):
        for m in range(P - d):
            wshift[d, m + d, m] = 1.0
    # bias fold: W_0 row 127 = ones reads the planted bias row; diag entries
    # for the stale rows 125/126 are cleared so they contribute nothing.
    wshift[0, 125, 125] = 0.0
    wshift[0, 126, 126] = 0.0
    wshift[0, 127, :] = 1.0
    biasinit = np.zeros((K - 1, F), dtype=np.float32)
    biasinit[K - 2, :] = bias
    return {
        "wrep": wrep,
        "biasinit": biasinit,
        "wshift": wshift,
        "biasrow": np.ascontiguousarray(bias[None, :], dtype=np.float32),
        "ones": np.ones((1, P), dtype=np.float32),
        "onesrow": np.ones((1, F), dtype=np.float32),
    }


# -------------------------------------------------------------------- main


def kernel(x, conv_state, weight, bias):
    global LAST_RESULTS
    from concourse.bass_utils import run_bass_kernel_spmd

    x = np.asarray(x, dtype=np.float32)
    conv_state = np.asarray(conv_state, dtype=np.float32)
    weight = np.asarray(weight, dtype=np.float32)
    bias = np.asarray(bias, dtype=np.float32)

    key = "nc_" + ALGO
    if key not in _CACHE:
        _CACHE[key] = _build_wino() if ALGO == "wino" else _build_shift()
    nc = _CACHE[key]

    consts = (
        _wino_consts_np(weight, bias)
        if ALGO == "wino"
        else _shift_consts_np(weight, bias)
    )
    in_maps = []
    for b in range(B):
        m = {
            "x": np.ascontiguousarray(x[b]),
            "state": np.ascontiguousarray(conv_state[b].T),
        }
        m.update(consts)
        in_maps.append(m)

    kwargs = {}
    if TRACE:
        kwargs = dict(trace=True, trace_cores=[0])
    res = run_bass_kernel_spmd(nc, in_maps, core_ids=list(range(B)), **kwargs)
    LAST_RESULTS = res

    out = np.stack([res.results[b]["out"] for b in range(B)], axis=0)
    new_conv_state = np.ascontiguousarray(
        x[:, S - (K - 1) :, :].transpose(0, 2, 1), dtype=np.float32
    )
    return out, new_conv_state
